# revision 19
# baseline (speedup 1.0000x reference)
"""BatchAllTripletLoss kernel for Trainium2 (8 NeuronCores, Bass/Tile).

Math shortcut: with labels = [0..N-1, 0..N-1], the (positive, negative)
mask of the [2N,2N,2N] triplet cube is nonzero only where the negative
index k is the same-label partner of the positive index j, i.e.
k = (j + N) mod 2N.  So the masked cube collapses to a [2N,2N] problem:

    t[i, j] = relu(d[i, j] - d[i, partner(j)] + 1)

Sharding: the [2N, 2N] = [512, 512] collapsed problem is tiled into 8
blocks of [128 anchors x 256 columns] where each core's column set
{128h..128h+128} u {128h+256..128h+384} is CLOSED under the partner
map (+N mod 2N), so u is computable core-locally.  128-row slabs use
all 128 SBUF partitions (2x the DVE/ACT throughput of a 64-row
sharding) and the matmuls stream only 256 columns.

Input diet: the batch is quantized to fp8 e3m4 on the host (4x fewer
HBM->SBUF bytes; the input DMA dominated the baseline), and the row
norms n2[j] = ||bq_j||^2 are computed ON THE HOST from the quantized
values, eliminating the on-device squares/norm chain.  The whole
distance epilogue is two fused instructions:

    PSUM   = hn2[j]                 (1-partition fp32 matmul, FIRST in
             the accumulation group: ones[1,128].T @ hn2[1,256]; runs
             while the fp8 chunks are still in flight)
    PSUM  += G[i,j] = <bq_i, bq_j>  (4 fp8 chunk matmuls)
    d[i,j] = Sqrt(-2*PSUM + bias_i) (one ACT op reading PSUM directly)

with hn2[j] = -(n2[j] + kappa/2)/2 and bias_i = n2[i] + kappa/2, i.e.
d = sqrt(n2i + n2j - 2G + kappa).  kappa = 1e-2 keeps the diagonal
sq_ii = kappa +- f32 accumulation noise (~1e-3) strictly positive, so
no max() clamp is needed; d_ii ~ 0.1 instead of the reference's 1e-7
shifts sum_sel by ~2e-4 relative (gate is 2e-2).  Host-side n2 from
the SAME quantized values makes sq_ii exact up to f32 rounding.

DMA plan (previous round's lesson: one queue serializes ~0.7us issues,
and a 128-descriptor broadcast DMA starved the chunk transfers):
  Sync:   hn2 row  [1,256]  (1 descriptor)  -> gates the n2 matmul
          chunk1   [128,2,384] fp8
  DVE:    chunk0   [128,2,384] fp8          -> gates the Gram matmuls
  GpSimd: bias     [128,1]   (tiny), then the const memsets
  ACT:    (framework table load,) dummy Sqrt to keep the lazy
          activation-table load off the critical path
Issues run on four sequencers in parallel; the 16 DMA engines see only
~200KB of real traffic.  No warm-up loop (the baseline's 25 fp32
warm-up matmuls monopolized the PE for ~10us and tripped the HAM 50%
clock throttle); three bf16 dummies just lift the PE off the cold
p-state.

Reductions per core (res[128,2] per-partition partials, host-summed):
    col 0: sum of relu(u - eps)
    col 1: count of entries with u > eps
Host combine: sum u*(u>eps) == sum relu(u-eps) + eps*count;
good = 2N^3 - CNT, bad = CNT (no u can equal f32(1e-5) exactly: u
lives on a 2^-19 grid).  mean(differences) of the antisymmetric cube
is exactly 0.  mean_norm_sq / rms come from the ORIGINAL f32 batch on
the host (exact).
"""

import os

import numpy as np

_TWO_N = 512  # 2N rows in the batch
_D = 512  # feature dim
_NCORES = 8
_A = 128  # anchor rows per core
_C = 256  # columns per core (closed under the partner map)
_KC = 128  # contraction chunk (partition dim)
_NK = _D // _KC  # 4 chunks
_T = _A + _C  # packed input columns: [anchors | colset]
_EPS_REL = 1e-5
_KAPPA = 1e-2  # diagonal positivity guard added to every sq entry

_NC_CACHE = None
LAST_RESULTS = None  # BassKernelResults of the most recent run (for profiling)


def _build_nc():
    import concourse.tile as tile
    from concourse import bacc, mybir

    f32 = mybir.dt.float32
    f8 = mybir.dt.float8e3  # e3m4: 4 mantissa bits, range +-15.5
    bf16 = mybir.dt.bfloat16
    AF = mybir.ActivationFunctionType
    ALU = mybir.AluOpType

    nc = bacc.Bacc("TRN2", target_bir_lowering=False, debug=False)
    # Two byte-packed input streams (fewer DMAs -> fewer 0.6us issue
    # slots, DGE delays, and completion-semaphore straggles):
    #   A[p, 0:768]     = bt chunks 0,1   (bt[p,k,t] = bq[sel[t], 128k+p],
    #                     fp8; t packs [128 anchor rows | 256 columns])
    #   A[0:2, 768:1280] = bf16 hi/lo limbs of -0.5*(n2[cols]+kappa/2)
    #                     (2-limb split keeps the norm row accurate to
    #                     ~1e-3 vs kappa=1e-2 in a single bf16 matmul)
    #   A[p, 1280:1284] = f32 bias[p] = n2[rows[p]] + kappa/2
    #   B[p, 0:768]     = bt chunks 2,3
    a_d = nc.dram_tensor("a", [_KC, 1284], mybir.dt.uint8, kind="ExternalInput")
    b_d = nc.dram_tensor("b", [_KC, 768], mybir.dt.uint8, kind="ExternalInput")
    res_d = nc.dram_tensor("res", [_A, 2], f32, kind="ExternalOutput")

    with tile.TileContext(nc) as tc:
        with (
            tc.tile_pool(name="sb", bufs=1) as sb,
            tc.tile_pool(name="ps", bufs=1, space="PSUM") as ps,
        ):
            # --- input DMAs: two streams, both on Sync -----------------
            # (ACT's sequencer is blocked early by the framework's 1.3us
            # activation-table load; GpSimd's adds branchy overhead.)
            tA = sb.tile([_KC, 1284], mybir.dt.uint8)
            nc.sync.dma_start(out=tA, in_=a_d.ap())
            tB = sb.tile([_KC, 768], mybir.dt.uint8)
            nc.sync.dma_start(out=tB, in_=b_d.ap())

            def lhsT(t, k):
                return t[:, 384 * k : 384 * k + _A].bitcast(f8)

            def rhs(t, k):
                return t[:, 384 * k + _A : 384 * (k + 1)].bitcast(f8)

            hn2_sb = tA[0:2, 768:1280].bitcast(bf16)  # [2, 256]
            bias_sb = tA[:, 1280:1284].bitcast(f32)  # [128, 1]

            # --- consts (DVE is idle until the u epilogue) -------------
            negeps = sb.tile([_A, 1], f32)
            nc.vector.memset(negeps, -_EPS_REL)
            red = sb.tile([_A, 2], f32)
            nc.vector.memset(red, 0.0)
            onesb = sb.tile([64, 64], bf16)
            nc.vector.memset(onesb, 1.0)
            ones2 = sb.tile([2, _A], bf16)
            nc.vector.memset(ones2, 1.0)
            dumm = sb.tile([1, 2], f32)
            nc.vector.memset(dumm, 1.0)

            # Preload the ACT Sqrt table during the DMA wait (a lazy
            # table load costs 1.3us on the critical path otherwise).
            nc.scalar.activation(dumm[:, 1:2], dumm[:, 0:1], AF.Sqrt)

            # Keep the PE busy through the DMA wait so its clock ramps
            # toward max by the time the real matmuls run (needs ~3us of
            # continuous activity); small bf16 dummies are cheap and too
            # brief to trip the HAM power throttle.
            warm_ps = ps.tile([64, 64], f32)
            for _ in range(10):
                nc.tensor.matmul(warm_ps, lhsT=onesb, rhs=onesb, start=True, stop=True)

            # PSUM = sum_k <bq_i, bq_j>_k + hn2_hi[j] + hn2_lo[j].  The
            # norm-limb matmul rides third: its input lands with stream A
            # (same semaphore as mm0/mm1), so nothing trails mm3.
            g_ps = ps.tile([_A, _C], f32)
            nc.tensor.matmul(g_ps, lhsT=lhsT(tA, 0), rhs=rhs(tA, 0), start=True, stop=False)
            nc.tensor.matmul(g_ps, lhsT=lhsT(tA, 1), rhs=rhs(tA, 1), start=False, stop=False)
            nc.tensor.matmul(g_ps, lhsT=ones2, rhs=hn2_sb, start=False, stop=False)
            nc.tensor.matmul(g_ps, lhsT=lhsT(tB, 0), rhs=rhs(tB, 0), start=False, stop=False)
            nc.tensor.matmul(g_ps, lhsT=lhsT(tB, 1), rhs=rhs(tB, 1), start=False, stop=True)

            # d = Sqrt(-2*PSUM + (n2_i + kappa/2)), straight from PSUM.
            dmat = sb.tile([_A, _C], f32)
            nc.scalar.activation(dmat, g_ps, AF.Sqrt, bias=bias_sb, scale=-2.0)

            # u[i,j] = d[i,j] + 1 - d[i, partner(j)]; partner swaps the
            # two 128-wide halves of the column set.
            H = _C // 2
            u = sb.tile([_A, _C], f32)
            nc.vector.scalar_tensor_tensor(
                out=u[:, 0:H],
                in0=dmat[:, 0:H],
                scalar=1.0,
                op0=ALU.add,
                in1=dmat[:, H:_C],
                op1=ALU.subtract,
            )
            nc.vector.scalar_tensor_tensor(
                out=u[:, H:_C],
                in0=dmat[:, H:_C],
                scalar=1.0,
                op0=ALU.add,
                in1=dmat[:, 0:H],
                op1=ALU.subtract,
            )

            # Two independent reductions run concurrently on DVE and ACT.
            gt = sb.tile([_A, _C], f32)
            nc.vector.tensor_scalar(
                out=gt,
                in0=u,
                scalar1=_EPS_REL,
                scalar2=None,
                op0=ALU.is_gt,
                op1=ALU.add,
                accum_out=red[:, 1:2],
            )
            relu = sb.tile([_A, _C], f32)
            nc.scalar.activation(
                relu,
                u,
                AF.Relu,
                bias=negeps,
                scale=1.0,
                accum_out=red[:, 0:1],
            )

            # Ship the per-partition partials; host does the final sums.
            # ACT's queue is untouched since the early table loads, so the
            # completion semaphore isn't queued behind retired traffic.
            nc.scalar.dma_start(out=res_d.ap(), in_=red, single_packet=True)

    nc.finalize()  # bacc register allocation + epilogue passes
    return nc


def _get_nc():
    global _NC_CACHE
    if _NC_CACHE is None:
        _NC_CACHE = _build_nc()
    return _NC_CACHE


def kernel(h1, h2, h3=None, **_unused):
    global LAST_RESULTS
    import ml_dtypes
    from concourse.bass_utils import run_bass_kernel_spmd

    h1 = np.ascontiguousarray(np.asarray(h1, dtype=np.float32))
    h2 = np.ascontiguousarray(np.asarray(h2, dtype=np.float32))
    batch = np.concatenate([h1, h2], axis=0)  # [2N, D]

    bq = batch.astype(ml_dtypes.float8_e3m4)
    # Row norms of the QUANTIZED batch (keeps the device-side diagonal
    # sq_ii = kappa up to f32 accumulation noise).
    n2 = (bq.astype(np.float64) ** 2).sum(axis=1)  # [2N] f64

    in_maps = []
    for c in range(_NCORES):
        a, h = c >> 1, c & 1
        rows = np.arange(_A * a, _A * a + _A)
        cols = np.r_[128 * h : 128 * h + 128, 128 * h + 256 : 128 * h + 384]
        sel = np.concatenate([rows, cols])  # [384]
        # bt[p, k, t] = bq[sel[t], 128k + p]
        bt = np.ascontiguousarray(
            bq[sel].T.reshape(_NK, _KC, _T).transpose(1, 0, 2)
        )
        btu = bt.view(np.uint8)  # [128, 4, 384]
        x = -0.5 * (n2[cols] + _KAPPA / 2)  # f64
        hi = x.astype(ml_dtypes.bfloat16)
        lo = (x - hi.astype(np.float64)).astype(ml_dtypes.bfloat16)
        hn2 = np.stack([hi, lo], axis=0)  # [2, C] bf16
        bias = (n2[rows] + _KAPPA / 2).astype(np.float32).reshape(_A, 1)
        A = np.zeros((_KC, 1284), dtype=np.uint8)
        A[:, 0:768] = btu[:, 0:2, :].reshape(_KC, 768)
        A[0:2, 768:1280] = np.ascontiguousarray(hn2).view(np.uint8)
        A[:, 1280:1284] = bias.view(np.uint8)
        B = np.ascontiguousarray(btu[:, 2:4, :].reshape(_KC, 768))
        in_maps.append({"a": A, "b": B})

    trace = os.environ.get("BASS_TRIPLET_TRACE", "0") == "1"
    kw = {}
    if trace:
        kw["trace"] = True
        kw["trace_cores"] = [
            int(x)
            for x in os.environ.get("BASS_TRIPLET_TRACE_CORES", "0").split(",")
        ]
        tmpdir = os.environ.get("BASS_TRIPLET_TMPDIR")
        if tmpdir:
            kw["tmpdir"] = tmpdir

    res = run_bass_kernel_spmd(_get_nc(), in_maps, core_ids=list(range(_NCORES)), **kw)
    LAST_RESULTS = res

    relu_sum = 0.0
    cnt_gt = 0.0
    for r in res.results:
        v = r["res"].astype(np.float64)  # [128, 2] per-partition partials
        relu_sum += float(v[:, 0].sum())
        cnt_gt += float(v[:, 1].sum())
    sum_sel = relu_sum + float(np.float32(_EPS_REL)) * cnt_gt

    mean_relevant = np.float32(sum_sel) / np.float32(cnt_gt)
    sum_n2_orig = float((batch.astype(np.float64) ** 2).sum())
    mean_norm_sq = np.float32(sum_n2_orig / _TWO_N)
    loss = np.float32(mean_relevant + np.float32(1e-4) * mean_norm_sq)
    mean_diff = np.float32(0.0)  # mean over the full antisymmetric cube is 0
    total = _TWO_N * _TWO_N * _TWO_N
    cnt_i = int(round(cnt_gt))
    good = np.int32(total - cnt_i)
    bad = np.int32(cnt_i)
    rms = np.float32(np.sqrt(mean_norm_sq))
    return (loss, mean_diff, good, bad, rms)
